# revision 8
# baseline (speedup 1.0000x reference)
"""CAM (channel attention) module kernel for Trainium2, 8 NeuronCores.

Reference computation (per batch b):
    q = x[b].reshape(C, N)                      # C=128, N=65536
    energy = q @ q.T                            # C x C
    att = softmax(rowmax(energy) - energy)      # == exp(rowmin(e)-e)/rowsum
    out = att @ q
    result = gamma * out + x

Sharding: every core takes the same N/8 = 8192 column slice of BOTH
batches.  Partial CxC energies are AllReduced across the 8 cores.

v2 design (transpose-first):
  - Load x as exact f32 (xf).  PE transposes the f32 chunks directly
    (2 cy/row = same PE cost as two fp16 transposes).
  - The fp16 hi/lo split for the energy matmul is produced FROM the
    transposed PSUM tiles: scalar casts PSUM->qT_hi (f16), vector
    subtracts PSUM-hi -> qT_lo (f16).  This fuses the casts with the
    PSUM drain that was needed anyway (halves DVE work vs v1).
  - Energy per 128-chunk: one 256-wide fp16 matmul accumulating
    [E_hh | C] (C = sum Qh_j Ql_j^T), E = E_hh + C + C^T.
  - AV matmul streams xf directly as float32r (1 cy/row at >=256 free
    dim) with attT (f16) as weights -- no fp16 copy of q needed.
  - The residual (+x, gamma folded into att) is accumulated ON the PE
    by a second matmul with an identity weight; output can be DMAed
    straight from PSUM to HBM (cfg flag) or staged via SBUF.
  - An optional tiny warm-up AllReduce issued at t=0 pulls the one-time
    collective barrier/setup off the critical path.
"""

import numpy as np

import concourse.bass as bass
import concourse.mybir as mybir
import concourse.tile as tile
from concourse import bacc
from concourse.bass_utils import run_bass_kernel_spmd
from concourse.masks import make_identity

B, C, D, H, W = 2, 128, 16, 64, 64
N = D * H * W  # 65536
NCORES = 8
NS = N // NCORES  # 8192 columns per core per batch

F32 = mybir.dt.float32
F32R = mybir.dt.float32r
F16 = mybir.dt.float16

# tuning knobs
CFG = dict(
    load_plan=(512, 512, 1024, 2048, 4096),
    warm_ar=True,       # tiny AllReduce at t=0 to prewarm the CC path
    fused_ar=False,     # single AllReduce for both batches vs 2 pipelined
    pe_residual=True,   # accumulate +x on the PE via identity matmul
    psum_store=False,   # DMA from PSUM is not supported by bass
    av_f32r=True,       # AV rhs = xf viewed as float32r
    avf=512,            # AV matmul free-dim chunk
    prefill=3,          # residual-matmul chunks issued before AR completes
    use_collective=True,
)

GROUPS = [[0, 1, 2, 3, 4, 5, 6, 7]]


def _body(nc: bass.Bass, tc: "tile.TileContext", xs, gm, out, cfg):
    AVF = cfg["avf"]
    JCH = NS // 128          # transposed 128-chunks per batch half (64)
    GB = 512                 # transpose group: 4 chunks
    gjp = GB // 128
    NG = NS // GB            # groups per batch (16)
    with (
        tc.tile_pool(name="big", bufs=1) as big,
        tc.tile_pool(name="small", bufs=1) as small,
        tc.tile_pool(name="work", bufs=3) as work,
        tc.tile_pool(name="psum_e", bufs=1, space="PSUM") as pse,
        tc.tile_pool(name="psum_av", bufs=3, space="PSUM") as psav,
        tc.tile_pool(name="trps", bufs=2, space="PSUM") as trps,
        tc.tile_pool(name="ctps", bufs=1, space="PSUM") as ctps,
        tc.tile_pool(name="dram", bufs=1, space="DRAM") as dram,
    ):
        # Persistent SBUF tensors; column range [b*NS, (b+1)*NS) = batch b
        xf = big.tile([C, 2 * NS], F32, tag="xf")      # exact f32 x
        # transposed chunks, [hi_j | lo_j] interleaved along the free dim
        qT = big.tile([128, 2 * JCH, 256], F16, tag="qT")

        # ---- warm-up AllReduce: absorbs the one-time barrier/setup ----
        if cfg["warm_ar"] and cfg["use_collective"]:
            wsb = small.tile([1, 1], F32, tag="wsb")
            nc.gpsimd.memset(wsb, 0.0)
            w_in = dram.tile([1, 1], F32, tag="w_in")
            w_out = dram.tile([1, 1], F32, tag="w_out")
            nc.sync.dma_start(w_in[:], wsb)
            nc.gpsimd.collective_compute(
                "AllReduce",
                mybir.AluOpType.add,
                replica_groups=GROUPS,
                ins=[w_in.opt()],
                outs=[w_out.opt()],
            )

        identh = small.tile([128, 128], F16, tag="identh")
        make_identity(nc, identh)
        ident = small.tile([128, 128], F32, tag="ident")
        make_identity(nc, ident)
        identr = small.tile([128, 128], F32, tag="identr")
        nc.scalar.copy(identr.bitcast(F32R), ident)

        g0 = small.tile([1, 1], F32, tag="g0")
        gsb = small.tile([128, 1], F32, tag="gsb")
        nc.sync.dma_start(g0[:], gm[None, :])
        nc.gpsimd.partition_broadcast(gsb, g0[:])

        ec_ps = [
            pse.tile([128, 256], F32, tag=f"ec_ps{b}", name=f"ec_ps{b}")
            for b in range(2)
        ]

        def load(b):
            pos = b * NS
            for i, ln in enumerate(cfg["load_plan"]):
                eng = nc.sync if i % 2 == 0 else nc.scalar
                eng.dma_start(xf[:, pos:pos + ln].bitcast(F32R),
                              xs[:, pos:pos + ln].bitcast(F32R))
                pos += ln
            assert pos == (b + 1) * NS

        def phase1(b):
            """f32 PE-transpose -> PSUM-side hi/lo cast -> energy MMs."""
            base = b * NS
            jbase = b * JCH

            def emit_emm(jlist):
                for j in jlist:
                    jj = jbase + j
                    nc.tensor.matmul(
                        ec_ps[b], lhsT=qT[:, jj, 0:128], rhs=qT[:, jj, :],
                        start=(j == 0), stop=(j == JCH - 1),
                    )

            for g in range(NG):
                tp = trps.tile([128, GB], F32, tag="tp")
                for u in range(gjp):
                    a0 = base + g * GB + u * 128
                    ps = slice(u * 128, (u + 1) * 128)
                    nc.tensor.transpose(
                        tp[:, ps].bitcast(F32R),
                        xf[:, a0:a0 + 128].bitcast(F32R),
                        identr.bitcast(F32R),
                    )
                jsl = slice(jbase + g * gjp, jbase + (g + 1) * gjp)
                tp3 = tp.rearrange("p (a b) -> p a b", b=128)
                # hi = fp16(xT)  (scalar casts while draining PSUM)
                nc.scalar.copy(qT[:, jsl, 0:128], tp3)
                # lo = fp16(xT - hi)
                nc.vector.tensor_tensor(
                    qT[:, jsl, 128:256], tp3, qT[:, jsl, 0:128],
                    mybir.AluOpType.subtract,
                )
                if g > 0:
                    emit_emm(range((g - 1) * gjp, g * gjp))
            emit_emm(range((NG - 1) * gjp, JCH))

        def partial_e(b):
            """e_sb = E_hh + C + C^T for this core's slice of batch b."""
            c_sb = small.tile([128, 128], F32, tag=f"c_sb{b}")
            nc.vector.tensor_copy(c_sb, ec_ps[b][:, 128:256])
            cT_ps = ctps.tile([128, 128], F32, tag="ctp")
            nc.tensor.transpose(cT_ps, c_sb, ident)
            e_sb = small.tile([128, 128], F32, tag=f"e_sb{b}")
            nc.vector.tensor_add(e_sb, ec_ps[b][:, 0:128], c_sb)
            nc.vector.tensor_add(e_sb, e_sb, cT_ps)
            return e_sb

        def reduce_energy(b, e_sb):
            """AllReduce one batch's partial energy across all 8 cores."""
            if not cfg["use_collective"]:
                return e_sb
            e_in = dram.tile([128, 128], F32, tag=f"e_in{b}")
            e_out = dram.tile([128, 128], F32, tag=f"e_out{b}")
            nc.sync.dma_start(e_in[:], e_sb)
            nc.gpsimd.collective_compute(
                "AllReduce",
                mybir.AluOpType.add,
                replica_groups=GROUPS,
                ins=[e_in.opt()],
                outs=[e_out.opt()],
            )
            e_full = small.tile([128, 128], F32, tag=f"e_full{b}")
            nc.sync.dma_start(e_full, e_out[:])
            return e_full

        def reduce_energy_fused(e0_sb, e1_sb):
            """Single AllReduce carrying both batches' partial energies."""
            if not cfg["use_collective"]:
                return e0_sb, e1_sb
            e_in = dram.tile([128, 256], F32, tag="e_in")
            e_out = dram.tile([128, 256], F32, tag="e_out")
            nc.sync.dma_start(e_in[:, 0:128], e0_sb)
            nc.sync.dma_start(e_in[:, 128:256], e1_sb)
            nc.gpsimd.collective_compute(
                "AllReduce",
                mybir.AluOpType.add,
                replica_groups=GROUPS,
                ins=[e_in.opt()],
                outs=[e_out.opt()],
            )
            ef = small.tile([128, 256], F32, tag="ef")
            nc.sync.dma_start(ef, e_out[:])
            return ef[:, 0:128], ef[:, 128:256]

        def softmax_attT(b, e_full):
            """att^T (fp16, gamma folded) from the reduced energy."""
            m = small.tile([128, 1], F32, tag=f"m{b}")
            nc.vector.tensor_reduce(
                m, e_full, axis=mybir.AxisListType.X, op=mybir.AluOpType.min
            )
            t = small.tile([128, 128], F32, tag=f"t{b}")
            r = small.tile([128, 1], F32, tag=f"r{b}")
            nc.scalar.activation(
                t, e_full, mybir.ActivationFunctionType.Exp,
                bias=m, scale=-1.0, accum_out=r,
            )
            rinv = small.tile([128, 1], F32, tag=f"rinv{b}")
            nc.vector.reciprocal(rinv, r)
            att = small.tile([128, 128], F32, tag=f"att{b}")
            nc.vector.tensor_scalar(
                att, t, rinv, gsb, mybir.AluOpType.mult, mybir.AluOpType.mult
            )
            attT_ps = ctps.tile([128, 128], F32, tag="ctp", name=f"attT_ps{b}")
            nc.tensor.transpose(attT_ps, att, ident)
            attT = small.tile([128, 128], F32, tag=f"attT{b}")
            nc.scalar.copy(attT.bitcast(F32R), attT_ps)
            return attT.bitcast(F32R)

        # ---- AV tail ----
        NCH = NS // AVF                      # chunks per batch (16)
        store_engs = [nc.sync, nc.scalar, nc.gpsimd]

        def av_rhs(sl):
            return xf[:, sl].bitcast(F32R)

        def resid_mm(av_ps, sl, start):
            # av_ps += I @ x  (the residual term, on the PE)
            nc.tensor.matmul(av_ps, lhsT=identr.bitcast(F32R), rhs=av_rhs(sl),
                             start=start, stop=False)

        def av_mm(av_ps, attT, sl, start):
            nc.tensor.matmul(av_ps, lhsT=attT, rhs=av_rhs(sl),
                             start=start, stop=True)

        def store(i, av_ps, sl):
            if cfg["psum_store"]:
                store_engs[i % 3].dma_start(out[:, sl], av_ps)
            else:
                o_sb = work.tile([128, AVF], F32, tag="o_sb")
                if i % 2 == 0:
                    nc.vector.tensor_copy(o_sb, av_ps)
                else:
                    nc.scalar.copy(o_sb, av_ps)
                store_engs[i % 3].dma_start(out[:, sl], o_sb)

        def chunk_slice(i):
            if cfg["fused_ar"]:
                # both attTs ready together: interleave batches
                b, k = i % 2, i // 2
            else:
                # pipelined ARs: all batch-0 chunks first
                b, k = i // NCH, i % NCH
            return b, slice(b * NS + k * AVF, b * NS + (k + 1) * AVF)

        av_tiles = {}

        def prefill(n):
            """Residual matmuls that don't depend on the AllReduce --
            keeps the PE warm through the AR wait, shortens the tail."""
            if not cfg["pe_residual"]:
                return
            for i in range(min(n, 2 * NCH)):
                b, sl = chunk_slice(i)
                av_ps = psav.tile([128, AVF], F32, tag="av_ps", name=f"av{i}")
                resid_mm(av_ps, sl, start=True)
                av_tiles[i] = av_ps

        def av_chunk(i, attT):
            b, sl = chunk_slice(i)
            if i in av_tiles:
                av_ps = av_tiles[i]
                av_mm(av_ps, attT, sl, start=False)
            else:
                av_ps = psav.tile([128, AVF], F32, tag="av_ps", name=f"av{i}")
                if cfg["pe_residual"]:
                    resid_mm(av_ps, sl, start=True)
                    av_mm(av_ps, attT, sl, start=False)
                else:
                    av_mm(av_ps, attT, sl, start=True)
            store(i, av_ps, sl)

        # ---- schedule over the two batches ----
        load(0)
        load(1)
        phase1(0)
        if cfg["fused_ar"]:
            e0_sb = partial_e(0)
            phase1(1)
            e1_sb = partial_e(1)
            e0, e1 = reduce_energy_fused(e0_sb, e1_sb)
            prefill(cfg["prefill"])
            a0 = softmax_attT(0, e0)
            a1 = softmax_attT(1, e1)
            for i in range(2 * NCH):
                av_chunk(i, a0 if i % 2 == 0 else a1)
        else:
            e0 = reduce_energy(0, partial_e(0))   # AR0 overlaps phase1(1)
            phase1(1)
            e1 = reduce_energy(1, partial_e(1))
            prefill(cfg["prefill"])
            a0 = softmax_attT(0, e0)
            for i in range(NCH):                  # tail 0 overlaps AR1
                av_chunk(i, a0)
            a1 = softmax_attT(1, e1)
            for i in range(NCH, 2 * NCH):
                av_chunk(i, a1)


_cached_nc = None


def _build(cfg=None):
    cfg = dict(CFG, **(cfg or {}))
    nc = bacc.Bacc(
        "TRN2",
        target_bir_lowering=False,
        debug=False,
        enable_asserts=False,
        num_devices=NCORES,
    )
    xs = nc.dram_tensor("xs", [C, 2 * NS], F32, kind="ExternalInput").ap()
    gm = nc.dram_tensor("gamma", [1], F32, kind="ExternalInput").ap()
    out = nc.dram_tensor("out", [C, 2 * NS], F32, kind="ExternalOutput").ap()
    with tile.TileContext(nc) as tc:
        _body(nc, tc, xs, gm, out, cfg)
    nc.compile()
    return nc


def kernel(x: np.ndarray, gamma: np.ndarray, _collect_results=None) -> np.ndarray:
    global _cached_nc
    if _cached_nc is None:
        _cached_nc = _build()
    nc = _cached_nc

    xr = np.ascontiguousarray(np.asarray(x, dtype=np.float32).reshape(B, C, N))
    gamma = np.ascontiguousarray(np.asarray(gamma, dtype=np.float32))
    in_maps = []
    for k in range(NCORES):
        shard = np.concatenate(
            [xr[0, :, k * NS:(k + 1) * NS], xr[1, :, k * NS:(k + 1) * NS]],
            axis=1,
        )
        in_maps.append({"xs": np.ascontiguousarray(shard), "gamma": gamma})

    res = run_bass_kernel_spmd(nc, in_maps, core_ids=list(range(NCORES)))
    if _collect_results is not None:
        _collect_results.append(res)

    outf = np.empty((B, C, N), np.float32)
    for k in range(NCORES):
        o = res.results[k]["out"]
        outf[0, :, k * NS:(k + 1) * NS] = o[:, :NS]
        outf[1, :, k * NS:(k + 1) * NS] = o[:, NS:]
    return outf.reshape(B, C, D, H, W)


# revision 9
# speedup vs baseline: 1.1870x; 1.1870x over previous
"""CAM (channel attention) module kernel for Trainium2, 8 NeuronCores.

Reference computation (per batch b):
    q = x[b].reshape(C, N)                      # C=128, N=65536
    energy = q @ q.T                            # C x C
    att = softmax(rowmax(energy) - energy)      # == exp(rowmin(e)-e)/rowsum
    out = att @ q
    result = gamma * out + x

Sharding: every core takes the same N/8 = 8192 column slice of BOTH
batches.  The two batches are pipelined: batch 0's energy -> AllReduce 0
(over all 8 cores) overlaps batch 1's energy compute, and batch 0's
AV/residual/store tail overlaps AllReduce 1.

Numerics: the PE matmuls run fp16 with an hi/lo split for the energy
term:  q = qh + ql (fp16 each, ~22 mantissa bits combined), and
    E = Qh Qh^T + C + C^T,   C = sum_j Qh_j Ql_j^T
which keeps the absolute error of the 65536-length dot products small
enough for the softmax (exp) stage.  The residual add uses the exact
f32 copy of x.  gamma is folded into the attention matrix.

v3: the post-AllReduce tail is the dominant serial cost (the first
collective syncs all cores behind a runtime barrier, so phase 1 is
mostly hidden); the tail is rebuilt for store throughput: residual adds
all on the vector engine, AV PSUM rotated over 6 banks, 1024-column
store staging round-robin over the DMA queues.
"""

import numpy as np

import concourse.bass as bass
import concourse.mybir as mybir
import concourse.tile as tile
from concourse import bacc
from concourse.bass_utils import run_bass_kernel_spmd
from concourse.masks import make_identity

B, C, D, H, W = 2, 128, 16, 64, 64
N = D * H * W  # 65536
NCORES = 8
NS = N // NCORES  # 8192 columns per core per batch

F32 = mybir.dt.float32
F16 = mybir.dt.float16

# tuning knobs
CFG = dict(
    nb=1024,          # pipeline block (cast/sub granularity)
    load_plan=(512, 512, 1024, 2048, 4096),
    load_2q=True,     # alternate load DMAs over sync+scalar queues
    store_nb=1024,    # output store DMA granularity
    avf=512,          # AV matmul free-dim chunk
    store_rot=3,      # number of store queues (2=hw only, 3=+gpsimd)
    use_collective=True,
)

GROUPS = [[0, 1, 2, 3, 4, 5, 6, 7]]


def _body(nc: bass.Bass, tc: "tile.TileContext", xs, gm, out, cfg):
    NB = cfg["nb"]
    AVF = cfg["avf"]
    JCH = NS // 128          # transposed 128-chunks per batch half
    with (
        tc.tile_pool(name="big", bufs=1) as big,
        tc.tile_pool(name="small", bufs=1) as small,
        tc.tile_pool(name="work", bufs=4) as work,
        tc.tile_pool(name="qlb", bufs=3) as qlb,
        tc.tile_pool(name="psum_e", bufs=1, space="PSUM") as pse,
        tc.tile_pool(name="psum_av", bufs=2, space="PSUM") as psav,
        tc.tile_pool(name="trps", bufs=2, space="PSUM") as trps,
        tc.tile_pool(name="dram", bufs=1, space="DRAM") as dram,
    ):
        # Persistent SBUF tensors; column range [b*NS, (b+1)*NS) = batch b
        xf = big.tile([C, 2 * NS], F32, tag="xf")      # exact f32 x
        qh = big.tile([C, 2 * NS], F16, tag="qh")      # fp16 hi (AV rhs)
        # transposed chunks, [hi_j | lo_j] interleaved along the free dim
        qT = big.tile([128, 2 * JCH, 256], F16, tag="qT")

        identh = small.tile([128, 128], F16, tag="identh")
        make_identity(nc, identh)
        ident = small.tile([128, 128], F32, tag="ident")
        make_identity(nc, ident)

        g0 = small.tile([1, 1], F32, tag="g0")
        gsb = small.tile([128, 1], F32, tag="gsb")
        nc.sync.dma_start(g0[:], gm[None, :])
        nc.gpsimd.partition_broadcast(gsb, g0[:])

        GB = 512
        gjp = GB // 128   # 4 chunks per transpose group

        ec_ps = [
            pse.tile([128, 256], F32, tag=f"ec_ps{b}", name=f"ec_ps{b}")
            for b in range(2)
        ]

        def load(b):
            pos = b * NS
            for i, ln in enumerate(cfg["load_plan"]):
                eng = nc.scalar if (cfg["load_2q"] and i % 2 == 1) else nc.sync
                eng.dma_start(xf[:, pos:pos + ln], xs[:, pos:pos + ln])
                pos += ln
            assert pos == (b + 1) * NS

        def phase1(b):
            """split-cast -> PE-transpose -> energy MMs for batch b."""
            base = b * NS
            jbase = b * JCH

            def emit_emm(jlist):
                for j in jlist:
                    jj = jbase + j
                    nc.tensor.matmul(
                        ec_ps[b], lhsT=qT[:, jj, 0:128], rhs=qT[:, jj, :],
                        start=(j == 0), stop=(j == JCH - 1),
                    )

            nblk = NS // NB
            for blk in range(nblk):
                sl = slice(base + blk * NB, base + (blk + 1) * NB)
                nc.vector.tensor_copy(qh[:, sl], xf[:, sl])        # fp16 hi
                ql = qlb.tile([C, NB], F16, tag="ql")
                nc.vector.tensor_tensor(                            # fp16 lo
                    ql, xf[:, sl], qh[:, sl], mybir.AluOpType.subtract
                )
                for gg in range(NB // GB):
                    g = blk * (NB // GB) + gg
                    th = trps.tile([128, GB], F16, tag="th")
                    tl = trps.tile([128, GB], F16, tag="tl")
                    for u in range(gjp):
                        a0 = base + blk * NB + gg * GB + u * 128
                        r0 = gg * GB + u * 128
                        ps = slice(u * 128, (u + 1) * 128)
                        nc.tensor.transpose(th[:, ps], qh[:, a0:a0 + 128], identh)
                        nc.tensor.transpose(tl[:, ps], ql[:, r0:r0 + 128], identh)
                    jsl = slice(jbase + g * gjp, jbase + (g + 1) * gjp)
                    nc.scalar.copy(
                        qT[:, jsl, 0:128],
                        th.rearrange("p (a b) -> p a b", b=128),
                    )
                    nc.vector.tensor_copy(
                        qT[:, jsl, 128:256],
                        tl.rearrange("p (a b) -> p a b", b=128),
                    )
                    if g > 0:
                        emit_emm(range((g - 1) * gjp, g * gjp))
            emit_emm(range(JCH - gjp, JCH))

        def reduce_energy(b):
            """E = E_hh + C + C^T, then AllReduce across all 8 cores."""
            c_sb = small.tile([128, 128], F32, tag=f"c_sb{b}")
            nc.vector.tensor_copy(c_sb, ec_ps[b][:, 128:256])
            cT_ps = trps.tile([128, 128], F32, tag="th")
            nc.tensor.transpose(cT_ps, c_sb, ident)
            e_sb = small.tile([128, 128], F32, tag=f"e_sb{b}")
            nc.vector.tensor_add(e_sb, ec_ps[b][:, 0:128], c_sb)
            nc.vector.tensor_add(e_sb, e_sb, cT_ps)
            if not cfg["use_collective"]:
                return e_sb
            e_in = dram.tile([128, 128], F32, tag=f"e_in{b}")
            e_out = dram.tile([128, 128], F32, tag=f"e_out{b}")
            nc.sync.dma_start(e_in[:], e_sb)
            nc.gpsimd.collective_compute(
                "AllReduce",
                mybir.AluOpType.add,
                replica_groups=GROUPS,
                ins=[e_in.opt()],
                outs=[e_out.opt()],
            )
            e_full = small.tile([128, 128], F32, tag=f"e_full{b}")
            nc.sync.dma_start(e_full, e_out[:])
            return e_full

        def softmax_attT(b, e_full):
            """att^T (fp16, gamma folded) from the reduced energy."""
            m = small.tile([128, 1], F32, tag=f"m{b}")
            nc.vector.tensor_reduce(
                m, e_full, axis=mybir.AxisListType.X, op=mybir.AluOpType.min
            )
            t = small.tile([128, 128], F32, tag=f"t{b}")
            r = small.tile([128, 1], F32, tag=f"r{b}")
            nc.scalar.activation(
                t, e_full, mybir.ActivationFunctionType.Exp,
                bias=m, scale=-1.0, accum_out=r,
            )
            rinv = small.tile([128, 1], F32, tag=f"rinv{b}")
            nc.vector.reciprocal(rinv, r)
            att = small.tile([128, 128], F16, tag=f"att{b}")
            nc.vector.tensor_scalar(
                att, t, rinv, gsb, mybir.AluOpType.mult, mybir.AluOpType.mult
            )
            attT_ps = trps.tile([128, 128], F16, tag="th", name=f"attT_ps{b}")
            nc.tensor.transpose(attT_ps, att, identh)
            attT = small.tile([128, 128], F16, tag=f"attT{b}")
            nc.scalar.copy(attT, attT_ps)
            return attT

        def av_tail(b, attT):
            """AV matmul + residual + store for batch b.

            Residual adds all on the vector engine (idle in the tail),
            AV PSUM rotated over 6 banks, stores at store_nb-column
            granularity round-robin over the DMA queues.
            """
            base = b * NS
            SNB = cfg["store_nb"]
            per_store = SNB // AVF
            store_engs = [nc.sync, nc.scalar, nc.gpsimd][:cfg["store_rot"]]
            nq = len(store_engs)
            o_sb = None
            for f in range(NS // AVF):
                sl = slice(base + f * AVF, base + (f + 1) * AVF)
                rr = f % 6
                if rr in (0, 1):
                    av_ps = psav.tile([128, AVF], F32, tag="av_ps",
                                      name=f"av{b}_{f}")
                elif rr == 2:
                    av_ps = trps.tile([128, AVF], F32, tag="th",
                                      name=f"avth{b}_{f}")
                elif rr == 3:
                    av_ps = trps.tile([128, AVF], F32, tag="tl",
                                      name=f"avtl{b}_{f}")
                else:
                    av_ps = pse.tile([128, AVF], F32, tag=f"ec_ps{rr - 4}",
                                     name=f"avec{b}_{f}")
                nc.tensor.matmul(av_ps, lhsT=attT, rhs=qh[:, sl],
                                 start=True, stop=True)
                if f % per_store == 0:
                    o_sb = work.tile([128, SNB], F32, tag="o_sb")
                osl = slice((f % per_store) * AVF, (f % per_store + 1) * AVF)
                nc.vector.tensor_add(o_sb[:, osl], av_ps, xf[:, sl])
                if (f + 1) % per_store == 0:
                    lo = (f + 1 - per_store) * AVF
                    hi = (f + 1) * AVF
                    dma_eng = store_engs[(f // per_store) % nq]
                    dma_eng.dma_start(out[:, base + lo:base + hi], o_sb)

        # ---- pipelined schedule over the two batches ----
        load(0)
        load(1)
        phase1(0)
        e0 = reduce_energy(0)      # AR0 overlaps phase1(1)
        phase1(1)
        e1 = reduce_energy(1)      # AR1 queues right behind AR0
        a0 = softmax_attT(0, e0)
        av_tail(0, a0)             # tail 0 overlaps AR1
        a1 = softmax_attT(1, e1)
        av_tail(1, a1)


_cached_nc = None


def _build(cfg=None):
    cfg = dict(CFG, **(cfg or {}))
    nc = bacc.Bacc(
        "TRN2",
        target_bir_lowering=False,
        debug=False,
        enable_asserts=False,
        num_devices=NCORES,
    )
    xs = nc.dram_tensor("xs", [C, 2 * NS], F32, kind="ExternalInput").ap()
    gm = nc.dram_tensor("gamma", [1], F32, kind="ExternalInput").ap()
    out = nc.dram_tensor("out", [C, 2 * NS], F32, kind="ExternalOutput").ap()
    with tile.TileContext(nc) as tc:
        _body(nc, tc, xs, gm, out, cfg)
    nc.compile()
    return nc


def kernel(x: np.ndarray, gamma: np.ndarray, _collect_results=None) -> np.ndarray:
    global _cached_nc
    if _cached_nc is None:
        _cached_nc = _build()
    nc = _cached_nc

    xr = np.ascontiguousarray(np.asarray(x, dtype=np.float32).reshape(B, C, N))
    gamma = np.ascontiguousarray(np.asarray(gamma, dtype=np.float32))
    in_maps = []
    for k in range(NCORES):
        shard = np.concatenate(
            [xr[0, :, k * NS:(k + 1) * NS], xr[1, :, k * NS:(k + 1) * NS]],
            axis=1,
        )
        in_maps.append({"xs": np.ascontiguousarray(shard), "gamma": gamma})

    res = run_bass_kernel_spmd(nc, in_maps, core_ids=list(range(NCORES)))
    if _collect_results is not None:
        _collect_results.append(res)

    outf = np.empty((B, C, N), np.float32)
    for k in range(NCORES):
        o = res.results[k]["out"]
        outf[0, :, k * NS:(k + 1) * NS] = o[:, :NS]
        outf[1, :, k * NS:(k + 1) * NS] = o[:, NS:]
    return outf.reshape(B, C, D, H, W)
